# revision 28
# baseline (speedup 1.0000x reference)
"""Causal self-attention (QKV GEMM + RoPE + causal softmax attention + output
projection) for Trainium2, sharded over 8 NeuronCores.

Sharding: tensor-parallel over heads (2 heads/core). Each core computes the
QKV projections for its heads (full token range), RoPE, causal attention, and
a partial output projection over its heads' channels; the host sums the 8
partial projections (the only cross-core reduction) and reshapes.

Matmul operands are fp16 (full-rate PE with hidden weight loads); all
accumulation is fp32 in PSUM, softmax statistics are fp32.

This version fuses the phases into one software-pipelined stream so the
PE never drains between them:
- phase A is emitted as 32 single-m-tile windows (16 QKV k-tiles each);
  RoPE / V-eviction / q,k-transposes of window m are deferred into window
  m+1 so they never stall the PE.
- attention is emitted as head-interleaved query-chunk pairs spliced
  between A windows as soon as their qkT tiles exist; the Scalar-bound
  exp work overlaps the PE-bound GEMM windows.
- softmax denominators: fp16 at-tile adds on DVE, one gpsimd
  partition_all_reduce per chunk (no PE, no extra PSUM), reciprocal and
  scale on DVE, pipelined one A-window behind.
- the output projection is spliced into the attention tail and reuses
  the A/B PSUM pools (everything fits the 8 banks).
- causal narrowing: on diagonal key tiles all ops run only on the valid
  [qlo:] columns, with one shared [128,128] triangle mask.
- all matmul operands are converted to fp16 on the host; x is laid out
  in DMA-issue order (contiguous per partition per window); output
  partials are fp16 and the host accumulates in fp32.
"""

import os
import sys

import numpy as np


def _ensure_concourse():
    try:
        import concourse.bass  # noqa: F401
        return
    except ImportError:
        pass
    for p in (
        "/opt/trn_rl_repo",
        os.path.expanduser("~/.axon_site/_ro/trn_rl_repo"),
        "/root/.axon_site/_ro/trn_rl_repo",
    ):
        if os.path.isdir(p) and p not in sys.path:
            sys.path.insert(0, p)
    import concourse.bass  # noqa: F401


# Problem shape (hardcoded per contract)
B, T, C, H = 2, 2048, 2048, 16
D, RD = 128, 64
NCORES = 8
HPC = H // NCORES          # heads per core = 2
BT = B * T                 # 4096
P = 128
MT = T // P                # 16 token tiles per batch
KTC = C // P               # 16 contraction tiles over C
FPC = 3 * HPC * D          # 768 qkv features per core
NQ = 512                   # query chunk
NJ = T // NQ               # 4 query chunks per instance
SCALE = 1.0 / float(np.sqrt(D))

_PROGRAM = None


def _build_program():
    _ensure_concourse()
    from collections import deque
    from contextlib import ExitStack

    import concourse.bacc as bacc
    import concourse.mybir as mybir
    import concourse.tile as tile
    from concourse import bass_isa
    from concourse.alu_op_type import AluOpType
    from concourse.masks import make_identity

    F32 = mybir.dt.float32
    MMDT = mybir.dt.float16
    EXP = mybir.ActivationFunctionType.Exp
    MUL = AluOpType.mult
    SUB = AluOpType.subtract
    ADD = AluOpType.add
    PSUM = "PSUM"

    nc = bacc.Bacc("TRN2", target_bir_lowering=False, debug=False,
                   num_devices=NCORES)

    xt_d = nc.dram_tensor("xt", [P, BT * KTC], MMDT, kind="ExternalInput").ap()
    w_d = nc.dram_tensor("wqkv", [P, KTC * FPC], MMDT, kind="ExternalInput").ap()
    cos_d = nc.dram_tensor("cosw", [P, (BT // P) * RD], F32, kind="ExternalInput").ap()
    sin_d = nc.dram_tensor("sinw", [P, (BT // P) * RD], F32, kind="ExternalInput").ap()
    msk_d = nc.dram_tensor("maskd", [P, P], MMDT, kind="ExternalInput").ap()
    wp_d = nc.dram_tensor("wproj", [P, HPC * C], MMDT, kind="ExternalInput").ap()
    out_d = nc.dram_tensor("outp", [BT, C], MMDT, kind="ExternalOutput").ap()
    # DRAM bounce rows for the softmax denominators: SBUF APs cannot have a
    # zero partition step, DRAM APs can, so the [1,NQ] reciprocal row round-
    # trips through DRAM to broadcast across partitions
    rbc_d = nc.dram_tensor("rbc", [B * NJ * HPC, NQ], MMDT,
                           kind="Internal").ap()

    WQ = KTC * FPC // 4        # qkv weight quarter, 4 k-tiles each

    with tile.TileContext(nc) as tc, ExitStack() as gctx:
        ep = gctx.enter_context

        const = ep(tc.tile_pool(name="const", bufs=1))
        msk_sb = const.tile([P, P], MMDT, tag="msk")
        cos_sb = const.tile([P, (BT // P) * RD], F32, tag="cos")
        sin_sb = const.tile([P, (BT // P) * RD], F32, tag="sin")
        ident = const.tile([P, P], MMDT, tag="ident")
        wp_sb = const.tile([P, HPC * C], MMDT, tag="wp")
        ones_col = const.tile([P, 1], MMDT, tag="onec")
        ones_row = const.tile([1, P], MMDT, tag="oner")

        make_identity(nc, ident[:])
        nc.vector.memset(ones_col[:], 1.0)
        nc.vector.memset(ones_row[:], 1.0)

        qkt_pool = ep(tc.tile_pool(name="qkt", bufs=2))
        v_pool = ep(tc.tile_pool(name="v", bufs=2))
        yt_pool = ep(tc.tile_pool(name="yt", bufs=1))
        yt_all = yt_pool.tile([P, B * HPC * T], MMDT, tag="yt")
        xcol = ep(tc.tile_pool(name="xcol", bufs=4))
        rotp = ep(tc.tile_pool(name="rot", bufs=3))
        tmpp = ep(tc.tile_pool(name="tmp", bufs=2))
        attnp = ep(tc.tile_pool(name="attn", bufs=10))
        saccp = ep(tc.tile_pool(name="sacc", bufs=4))
        rowp = ep(tc.tile_pool(name="rows", bufs=4))
        rrepp = ep(tc.tile_pool(name="rrep", bufs=2))
        outrow = ep(tc.tile_pool(name="orow", bufs=3))

        # PSUM: exactly 8 banks
        ps5 = ep(tc.tile_pool(name="ps5", bufs=2, space=PSUM))   # qk gemm
        ps2 = ep(tc.tile_pool(name="ps2", bufs=1, space=PSUM))   # v gemm
        pst = ep(tc.tile_pool(name="pst", bufs=1, space=PSUM))   # transposes
        pss = ep(tc.tile_pool(name="pss", bufs=2, space=PSUM))   # scores
        psy = ep(tc.tile_pool(name="psy", bufs=2, space=PSUM))   # attn out

        # x chunk prefetching, one [P, KTC, P] chunk per A window
        prefetched = {}

        def fetch_x(b, m, split=1):
            key = (b, m)
            if key in prefetched:
                return prefetched.pop(key)
            xo = (b * MT + m) * KTC * P
            xc = xcol.tile([P, KTC, P], MMDT, tag="xc")
            # issue on the Act queue: the gpsimd queue carries the long
            # partition_all_reduce calls and must not delay x loads
            kc = KTC // split
            for s in range(split):
                nc.scalar.dma_start(
                    out=xc[:, s * kc:(s + 1) * kc, :],
                    in_=xt_d[:, xo + s * kc * P:xo + (s + 1) * kc * P]
                    .rearrange("p (k t) -> p k t", k=kc))
            return xc

        def prefetch_x(b, m, split=1):
            prefetched[(b, m)] = fetch_x(b, m, split=split)

        # first window split into 4 sub-DMAs so the first matmul only
        # waits for a quarter of the window (plus the first w chunk)
        prefetch_x(0, 0, split=4)

        wstack = ExitStack()
        wpool = wstack.enter_context(tc.tile_pool(name="wqkv", bufs=1))
        w_sbs = [wpool.tile([P, WQ], MMDT, tag=f"w{q}", name=f"w{q}")
                 for q in range(4)]
        # per-ktile weight chunks: each sweep matmul depends on a ~200KB
        # transfer, not a whole 0.8MB quarter — the DMA-bound first
        # windows stream instead of stalling at quarter boundaries
        for q in range(4):
            for c4 in range(4):
                nc.sync.dma_start(
                    out=w_sbs[q][:, c4 * FPC:(c4 + 1) * FPC],
                    in_=w_d[:, q * WQ + c4 * FPC:q * WQ + (c4 + 1) * FPC])
        # consts after the weights on the SP queue; cos/sin are consumed
        # one [P,RD] slice per window, so stream them in quarters to keep
        # them out of the startup HBM burst
        CS4 = (BT // P) * RD // 4
        for c4 in range(4):
            nc.sync.dma_start(out=cos_sb[:, c4 * CS4:(c4 + 1) * CS4],
                              in_=cos_d[:, c4 * CS4:(c4 + 1) * CS4])
            nc.sync.dma_start(out=sin_sb[:, c4 * CS4:(c4 + 1) * CS4],
                              in_=sin_d[:, c4 * CS4:(c4 + 1) * CS4])
            if c4 == 0:
                nc.sync.dma_start(out=msk_sb[:], in_=msk_d)
        nc.sync.dma_start(out=wp_sb[:], in_=wp_d)

        def wslice(kt, lo, hi):
            return w_sbs[kt // 4][:, (kt % 4) * FPC + lo:(kt % 4) * FPC + hi]

        qkts = {}
        v_sbs = {}
        # attention work broken into per-tile quanta, pumped one per
        # matmul slot inside the A windows / C tiles so the Scalar-bound
        # exp stream overlaps the PE-bound GEMMs instead of serializing
        quanta = deque()

        def pump(n=1):
            for _ in range(n):
                if not quanta:
                    return
                quanta.popleft()()

        # deferred per-window epilogues:
        # - V eviction of window g lands in window g+1 (depends only on
        #   the p2 psum, never on the Vector queue)
        # - the rot transposes of window g land in window g+2, so a full
        #   window of Vector-queue lag on RoPE never stalls the PE
        pending_v = [None]
        pending_t = deque()

        def flush_v():
            if pending_v[0] is None:
                return
            b, m, p2 = pending_v[0]
            pending_v[0] = None
            nc.scalar.copy(v_sbs[b][:, m * HPC * D:(m + 1) * HPC * D], p2[:])

        def flush_t(force=False):
            while pending_t and (force or len(pending_t) >= 2):
                b, m, rot = pending_t.popleft()
                tp4 = pst.tile([P, 4, P], MMDT, tag="tp", name=f"tp_{b}_{m}")
                for hb in range(4):
                    nc.tensor.transpose(tp4[:, hb, :],
                                        rot[:, hb * P:(hb + 1) * P], ident[:])
                qv = qkts[b][:].rearrange("p (hb t) -> p hb t", hb=4)
                nc.scalar.copy(qv[:, :, m * P:(m + 1) * P], tp4[:])

        def emit_a(b, m):
            if m == 0:
                qkts[b] = qkt_pool.tile([P, 4 * T], MMDT, tag="qkT",
                                        name=f"qkT_{b}")
                v_sbs[b] = v_pool.tile([P, MT * HPC * D], MMDT, tag="v",
                                       name=f"v_{b}")
            gi = b * MT + m
            # keep two windows of x in flight: the Act queue can lag
            # behind a pair's exp backlog
            for ahead in (1, 2):
                ni = gi + ahead
                if ni < B * MT and (ni // MT, ni % MT) not in prefetched \
                        and ni != gi:
                    prefetch_x(ni // MT, ni % MT)
            xc = fetch_x(b, m)
            p5 = ps5.tile([P, 512], F32, tag="p5", name=f"p5_{b}_{m}")
            p2 = ps2.tile([P, 256], F32, tag="p2", name=f"p2_{b}_{m}")
            # no pumps in this sweep: pumped score matmuls depend on the
            # qkT copies emitted by flush_t below, and must stay behind
            # them in the PE queue order
            for kt in range(KTC):
                nc.tensor.matmul(p5[:], xc[:, kt, :], wslice(kt, 0, 512),
                                 start=(kt == 0), stop=(kt == KTC - 1))
            flush_v()
            flush_t()
            # RoPE on the q|k half, writes rot
            gm = b * MT + m
            rot = rotp.tile([P, 512], MMDT, tag="rot", name=f"rot_{b}_{m}")
            p3 = p5[:].rearrange("p (blk two d) -> p blk two d", two=2, d=RD)
            re_, im_ = p3[:, :, 0, :], p3[:, :, 1, :]
            r3 = rot[:].rearrange("p (blk two d) -> p blk two d", two=2, d=RD)
            cosb = (cos_sb[:, gm * RD:(gm + 1) * RD]
                    .unsqueeze(1).broadcast_to([P, 4, RD]))
            sinb = (sin_sb[:, gm * RD:(gm + 1) * RD]
                    .unsqueeze(1).broadcast_to([P, 4, RD]))
            t1 = tmpp.tile([P, 256], F32, tag="t1")
            t2 = tmpp.tile([P, 256], F32, tag="t2")
            t1v = t1[:].rearrange("p (blk d) -> p blk d", d=RD)
            t2v = t2[:].rearrange("p (blk d) -> p blk d", d=RD)
            # the products must read the p5 PSUM, so they stay on Vector;
            # the combining SUB/ADD are SBUF->SBUF (and were the slow
            # strided-fp16-write ops), so they run on the otherwise-idle
            # gpsimd engine — this takes the Vector queue off the
            # window-seam critical path.
            nc.vector.tensor_tensor(t1v, re_, cosb, MUL)
            nc.vector.tensor_tensor(t2v, im_, sinb, MUL)
            nc.gpsimd.tensor_tensor(r3[:, :, 0, :], t1v, t2v, SUB)
            t3 = tmpp.tile([P, 256], F32, tag="t3")
            t4 = tmpp.tile([P, 256], F32, tag="t4")
            t3v = t3[:].rearrange("p (blk d) -> p blk d", d=RD)
            t4v = t4[:].rearrange("p (blk d) -> p blk d", d=RD)
            nc.vector.tensor_tensor(t3v, re_, sinb, MUL)
            nc.vector.tensor_tensor(t4v, im_, cosb, MUL)
            nc.gpsimd.tensor_tensor(r3[:, :, 1, :], t3v, t4v, ADD)
            # V projection sweep after the eviction so ps2 (bufs=1) is free
            for kt in range(KTC):
                nc.tensor.matmul(p2[:], xc[:, kt, :], wslice(kt, 512, FPC),
                                 start=(kt == 0), stop=(kt == KTC - 1))
                pump()
            pending_v[0] = (b, m, p2)
            pending_t.append((b, m, rot))

        # ---- attention chunk pairs (both heads), as pumpable quanta ----
        chunk_st = {}

        def enqueue_p(b, j):
            qkT = qkts[b]
            v_sb = v_sbs[b]
            nkt = 4 * (j + 1)
            st = {"y": {}, "sacc": {}, "at": {}}
            chunk_st[(b, j)] = st

            def q_score(kt, h, ktl, qlo):
                def f():
                    if kt == 0:
                        st["y"][h] = psy.tile([P, NQ], F32, tag="y",
                                              name=f"y_{b}_{j}_{h}")
                        st["sacc"][h] = saccp.tile([P, NQ], MMDT, tag="sa",
                                                   name=f"sa_{b}_{j}_{h}")
                    sacc = st["sacc"][h]
                    sc = pss.tile([P, NQ], F32, tag="sc",
                                  name=f"sc_{b}_{j}_{kt}_{h}")
                    nc.tensor.matmul(
                        sc[:, qlo:],
                        qkT[:, (2 + h) * T + kt * P:
                            (2 + h) * T + (kt + 1) * P],
                        qkT[:, h * T + j * NQ + qlo: h * T + (j + 1) * NQ],
                        start=True, stop=True)
                    at = attnp.tile([P, NQ], MMDT, tag="at",
                                    name=f"at_{b}_{j}_{kt}_{h}")
                    nc.scalar.activation(at[:, qlo:], sc[:, qlo:], EXP,
                                         scale=SCALE)
                    if ktl >= 0:
                        nc.vector.tensor_tensor(
                            at[:, qlo:qlo + P], at[:, qlo:qlo + P],
                            msk_sb[:], MUL)
                    if kt == 0:
                        nc.vector.tensor_copy(sacc[:], at[:])
                    else:
                        nc.vector.tensor_tensor(sacc[:, qlo:], sacc[:, qlo:],
                                                at[:, qlo:], ADD)
                    st["at"][(kt, h)] = at
                return f

            def q_v(kt, h, qlo):
                def f():
                    at = st["at"].pop((kt, h))
                    nc.tensor.matmul(
                        st["y"][h][:, qlo:],
                        v_sb[:, kt * HPC * D + h * D:
                             kt * HPC * D + (h + 1) * D],
                        at[:, qlo:], start=(kt == 0), stop=(kt == nkt - 1),
                        skip_group_check=True)
                return f

            def q_f1(h):
                # softmax denominator via the PE: a ones-column matmul
                # sums sacc over its key partitions into a [1,NQ] psum
                # row (213ns) — unlike the 3.5us gpsimd
                # partition_all_reduce, this never turns into a long
                # cross-engine stall when the list scheduler hoists the
                # downstream reciprocal
                def f():
                    st.setdefault("rrep", {})
                    prow = pss.tile([P, NQ], F32, tag="sc",
                                    name=f"dsum_{b}_{j}_{h}")
                    nc.tensor.matmul(prow[0:1, :], ones_col[:],
                                     st["sacc"][h][:], start=True, stop=True)
                    srow = rowp.tile([1, NQ], F32, tag="dr",
                                     name=f"drow_{b}_{j}_{h}")
                    nc.scalar.copy(srow[:], prow[0:1, :])
                    rrow32 = rowp.tile([1, NQ], F32, tag="rrow32",
                                       name=f"rrow32_{b}_{j}_{h}")
                    with nc.allow_low_precision(reason="softmax recip"):
                        nc.vector.reciprocal_approx_fast(out=rrow32[:],
                                                         in_=srow[:])
                    rrow = rowp.tile([1, NQ], MMDT, tag="rrow",
                                     name=f"rrow_{b}_{j}_{h}")
                    nc.scalar.copy(rrow[:], rrow32[:])
                    # DRAM bounce: broadcast the [1,NQ] reciprocal row
                    # across partitions (SBUF APs cannot have zero
                    # partition step, DRAM APs can). Done here, windows
                    # before F2 consumes it, so the DMA latency never
                    # touches the psy bank handoff.
                    ri = (b * NJ + j) * HPC + h
                    nc.sync.dma_start(out=rbc_d[ri:ri + 1, :], in_=rrow[:])
                    rrep = rrepp.tile([P, NQ], MMDT, tag="rr",
                                      name=f"rr_{b}_{j}_{h}")
                    nc.sync.dma_start(
                        out=rrep[:],
                        in_=rbc_d[ri:ri + 1, :].broadcast_to([P, NQ]))
                    st["rrep"][h] = rrep
                return f

            # the v matmuls trail their score quanta by VLAG k-tiles: the
            # exp -> mask -> sacc chain for a tile then has ~2.5us of
            # slack before the PE's y matmul needs the at tile, so a
            # backlogged Scalar queue no longer stalls the PE
            VLAG = 3

            def qlo_of(kt):
                return max(kt - (nkt - 4), 0) * P

            for kt in range(nkt):
                ktl = kt - (nkt - 4)
                qlo = qlo_of(kt)
                for h in range(HPC):
                    quanta.append(q_score(kt, h, ktl, qlo))
                if kt >= VLAG:
                    for h in range(HPC):
                        quanta.append(q_v(kt - VLAG, h, qlo_of(kt - VLAG)))
            for kt in range(max(nkt - VLAG, 0), nkt):
                for h in range(HPC):
                    quanta.append(q_v(kt, h, qlo_of(kt)))
            for h in range(HPC):
                quanta.append(q_f1(h))

        def emit_f2(b, j):
            # normalization: reciprocal on the [1,NQ] denominator row,
            # broadcast across partitions with a rank-1 ones outer
            # product on the PE, then scale y. Every link is a short op
            # with multi-window slack, so nothing here can stall a queue.
            st = chunk_st.pop((b, j))
            inst0 = b * HPC
            for h in range(HPC):
                rrep = st["rrep"][h]
                nc.vector.tensor_tensor(
                    yt_all[:, (inst0 + h) * T + j * NQ:
                           (inst0 + h) * T + (j + 1) * NQ],
                    st["y"][h][:], rrep[:], MUL)

        # ---- output projection m-tile (reuses the A/B psum banks) ----
        c_cnt = [0]

        def emit_c(b, m):
            orow = outrow.tile([P, C], MMDT, tag="orow")
            for oc in range(4):
                pool, tg = (ps5, "p5") if c_cnt[0] % 2 == 0 else (pss, "sc")
                c_cnt[0] += 1
                op = pool.tile([P, 512], F32, tag=tg, name=f"op_{b}_{m}_{oc}")
                for h in range(HPC):
                    nc.tensor.matmul(
                        op[:],
                        yt_all[:, (b * HPC + h) * T + m * P:
                               (b * HPC + h) * T + (m + 1) * P],
                        wp_sb[:, h * C + oc * 512: h * C + (oc + 1) * 512],
                        start=(h == 0), stop=(h == HPC - 1))
                if oc < 3:
                    pump()
                # single-shot [P,512] evictions, balanced so Scalar keeps
                # headroom for the last pair's exp stream: Vector takes
                # oc0, oc1 and the tail half of oc2; Scalar takes oc3 and
                # the head half of oc2
                if oc in (0, 1):
                    nc.vector.tensor_copy(
                        orow[:, oc * 512:(oc + 1) * 512], op[:])
                elif oc == 2:
                    nc.scalar.copy(orow[:, oc * 512:oc * 512 + 256],
                                   op[:, 0:256])
                    nc.vector.tensor_copy(
                        orow[:, oc * 512 + 256:(oc + 1) * 512],
                        op[:, 256:512])
                else:
                    nc.scalar.copy(orow[:, oc * 512:(oc + 1) * 512], op[:])
                if b == 1 and m >= 14:
                    # last tiles: per-oc DMA chunks so the final transfer
                    # overlaps the remaining evictions instead of
                    # lengthening the tail
                    nc.sync.dma_start(
                        out=out_d[(b * MT + m) * P:(b * MT + m + 1) * P,
                                  oc * 512:(oc + 1) * 512],
                        in_=orow[:, oc * 512:(oc + 1) * 512])
            if not (b == 1 and m >= 14):
                nc.sync.dma_start(
                    out=out_d[(b * MT + m) * P:(b * MT + m + 1) * P, :],
                    in_=orow[:])

        # ---- the fused schedule ----
        # pair j's quanta (including its two trailing F1 reduces) enqueue
        # after window g_P = 16b+4(j+1) and drain one per p2-loop matmul
        # slot in windows g_P+1..g_P+4 (16 slots each, 16j+18 quanta). F2
        # lands at window g_P+8: its Vector-queue reciprocal must only run
        # after the 3.5us gpsimd reduces, or it stalls the RoPE ops queued
        # behind it (which the flush_t transposes wait on).
        f2_at = {}
        for b in range(B):
            for j in range(4):
                f2_at.setdefault(b * MT + 4 * (j + 1) + 8, []).append((b, j))
        sched = []
        for b in range(B):
            for m in range(MT):
                g = b * MT + m
                for (fb, fj) in f2_at.get(g, []):
                    sched.append(("F2", fb, fj))
                sched.append(("A", b, m))
                if m in (4, 8, 12):
                    sched.append(("P", b, m // 4 - 1))
                if b > 0 and m == 0:
                    sched.append(("P", b - 1, 3))
        bl = B - 1

        emitters = {"A": emit_a, "P": enqueue_p, "F2": emit_f2, "C": emit_c}
        for kind, b, i in sched:
            emitters[kind](b, i)

        # ---- tail: flush the last epilogues, drain pair (bl,3) through
        # the C tiles' pump slots ----
        flush_v()
        flush_t(force=True)
        emit_f2(bl, 1)          # g_P+8 = 32 lands here
        enqueue_p(bl, 3)
        emit_c(0, 0)
        emit_f2(bl, 2)          # its F1 reduces drained in C(0,0)'s pumps
        for m in range(1, MT):
            emit_c(0, m)
        for m in range(0, 8):
            emit_c(1, m)
        pump(len(quanta))       # leftovers of (bl,3), incl. its F1s
        emit_f2(bl, 3)
        for m in range(8, MT):
            emit_c(1, m)

        wstack.close()

    nc.compile()
    return nc


def _perm(rows):
    return np.concatenate([rows[0::2], rows[1::2]], axis=0)


def _host_inputs(x, mask, freqs_cos, freqs_sin, w_attn, w_proj):
    f32 = np.float32
    f16 = np.float16
    x = np.asarray(x, f32)
    fc = np.asarray(freqs_cos, f32)
    fs = np.asarray(freqs_sin, f32)
    w_attn = np.asarray(w_attn, f32)
    w_proj = np.asarray(w_proj, f32)

    # x in DMA-issue order: per partition, contiguous [b][m][kt][tok]
    Xv = x.reshape(B, MT, P, KTC, P).transpose(4, 0, 1, 3, 2)
    # [p, b, m, kt, tok]
    xt_host = np.ascontiguousarray(Xv.reshape(P, -1)).astype(f16)

    def rows_arrange(a):  # [BT, RD] -> [P, (BT//P)*RD]
        return np.ascontiguousarray(
            a.reshape(BT // P, P, RD).transpose(1, 0, 2).reshape(P, -1))

    cosw = rows_arrange(np.concatenate([fc] * B, axis=0))
    sinw = rows_arrange(np.concatenate([fs] * B, axis=0))

    # one [k, q] triangle (attend iff k <= q) covers every diagonal subtile
    maskd = np.ascontiguousarray(np.triu(np.ones((P, P), dtype=f16)))

    wq, wk, wv = w_attn[0:C], w_attn[C:2 * C], w_attn[2 * C:3 * C]
    in_maps = []
    for c in range(NCORES):
        h0, h1 = HPC * c, HPC * c + 1
        Wc = np.concatenate([
            _perm(wq[h0 * D:(h0 + 1) * D]), _perm(wq[h1 * D:(h1 + 1) * D]),
            _perm(wk[h0 * D:(h0 + 1) * D]), _perm(wk[h1 * D:(h1 + 1) * D]),
            wv[h0 * D:(h0 + 1) * D], wv[h1 * D:(h1 + 1) * D]], axis=0)
        wqkv_c = np.ascontiguousarray(
            Wc.T.reshape(KTC, P, FPC).transpose(1, 0, 2).reshape(P, KTC * FPC)
        ).astype(f16)
        wp_c = w_proj[:, c * HPC * D:(c + 1) * HPC * D].T  # [256, C]
        wp_c = np.ascontiguousarray(
            wp_c.reshape(HPC, P, C).transpose(1, 0, 2).reshape(P, HPC * C)
        ).astype(f16)
        in_maps.append({
            "xt": xt_host, "wqkv": wqkv_c, "cosw": cosw, "sinw": sinw,
            "maskd": maskd, "wproj": wp_c,
        })
    return in_maps


def kernel(x, mask, freqs_cos, freqs_sin, w_attn, w_proj):
    global _PROGRAM
    _ensure_concourse()
    from concourse.bass_utils import run_bass_kernel_spmd

    if _PROGRAM is None:
        _PROGRAM = _build_program()
    nc = _PROGRAM

    in_maps = _host_inputs(x, mask, freqs_cos, freqs_sin, w_attn, w_proj)
    res = run_bass_kernel_spmd(nc, in_maps, list(range(NCORES)))
    out = res.results[0]["outp"].astype(np.float32)
    for i in range(1, NCORES):
        out = out + res.results[i]["outp"].astype(np.float32)
    return np.ascontiguousarray(out.reshape(B, T, C))



# revision 29
# speedup vs baseline: 1.0104x; 1.0104x over previous
"""Causal self-attention (QKV GEMM + RoPE + causal softmax attention + output
projection) for Trainium2, sharded over 8 NeuronCores.

Sharding: tensor-parallel over heads (2 heads/core). Each core computes the
QKV projections for its heads (full token range), RoPE, causal attention, and
a partial output projection over its heads' channels; the host sums the 8
partial projections (the only cross-core reduction) and reshapes.

Matmul operands are fp16 (full-rate PE with hidden weight loads); all
accumulation is fp32 in PSUM, softmax statistics are fp32.

This version fuses the phases into one software-pipelined stream so the
PE never drains between them:
- phase A is emitted as 32 single-m-tile windows (16 QKV k-tiles each);
  RoPE / V-eviction / q,k-transposes of window m are deferred into window
  m+1 so they never stall the PE.
- attention is emitted as head-interleaved query-chunk pairs spliced
  between A windows as soon as their qkT tiles exist; the Scalar-bound
  exp work overlaps the PE-bound GEMM windows.
- softmax denominators: fp16 at-tile adds on DVE, one gpsimd
  partition_all_reduce per chunk (no PE, no extra PSUM), reciprocal and
  scale on DVE, pipelined one A-window behind.
- the output projection is spliced into the attention tail and reuses
  the A/B PSUM pools (everything fits the 8 banks).
- causal narrowing: on diagonal key tiles all ops run only on the valid
  [qlo:] columns, with one shared [128,128] triangle mask.
- all matmul operands are converted to fp16 on the host; x is laid out
  in DMA-issue order (contiguous per partition per window); output
  partials are fp16 and the host accumulates in fp32.
"""

import os
import sys

import numpy as np


def _ensure_concourse():
    try:
        import concourse.bass  # noqa: F401
        return
    except ImportError:
        pass
    for p in (
        "/opt/trn_rl_repo",
        os.path.expanduser("~/.axon_site/_ro/trn_rl_repo"),
        "/root/.axon_site/_ro/trn_rl_repo",
    ):
        if os.path.isdir(p) and p not in sys.path:
            sys.path.insert(0, p)
    import concourse.bass  # noqa: F401


# Problem shape (hardcoded per contract)
B, T, C, H = 2, 2048, 2048, 16
D, RD = 128, 64
NCORES = 8
HPC = H // NCORES          # heads per core = 2
BT = B * T                 # 4096
P = 128
MT = T // P                # 16 token tiles per batch
KTC = C // P               # 16 contraction tiles over C
FPC = 3 * HPC * D          # 768 qkv features per core
NQ = 512                   # query chunk
NJ = T // NQ               # 4 query chunks per instance
SCALE = 1.0 / float(np.sqrt(D))

_PROGRAM = None


def _build_program():
    _ensure_concourse()
    from collections import deque
    from contextlib import ExitStack

    import concourse.bacc as bacc
    import concourse.mybir as mybir
    import concourse.tile as tile
    from concourse import bass_isa
    from concourse.alu_op_type import AluOpType
    from concourse.masks import make_identity

    F32 = mybir.dt.float32
    MMDT = mybir.dt.float16
    EXP = mybir.ActivationFunctionType.Exp
    MUL = AluOpType.mult
    SUB = AluOpType.subtract
    ADD = AluOpType.add
    PSUM = "PSUM"

    nc = bacc.Bacc("TRN2", target_bir_lowering=False, debug=False,
                   num_devices=NCORES)

    xt_d = nc.dram_tensor("xt", [P, BT * KTC], MMDT, kind="ExternalInput").ap()
    w_d = nc.dram_tensor("wqkv", [P, KTC * FPC], MMDT, kind="ExternalInput").ap()
    cos_d = nc.dram_tensor("cosw", [P, (BT // P) * RD], F32, kind="ExternalInput").ap()
    sin_d = nc.dram_tensor("sinw", [P, (BT // P) * RD], F32, kind="ExternalInput").ap()
    msk_d = nc.dram_tensor("maskd", [P, P], MMDT, kind="ExternalInput").ap()
    wp_d = nc.dram_tensor("wproj", [P, HPC * C], MMDT, kind="ExternalInput").ap()
    out_d = nc.dram_tensor("outp", [BT, C], MMDT, kind="ExternalOutput").ap()
    # DRAM bounce rows for the softmax denominators: SBUF APs cannot have a
    # zero partition step, DRAM APs can, so the [1,NQ] reciprocal row round-
    # trips through DRAM to broadcast across partitions
    rbc_d = nc.dram_tensor("rbc", [B * NJ * HPC, NQ], MMDT,
                           kind="Internal").ap()

    WQ = KTC * FPC // 4        # qkv weight quarter, 4 k-tiles each

    with tile.TileContext(nc) as tc, ExitStack() as gctx:
        ep = gctx.enter_context

        const = ep(tc.tile_pool(name="const", bufs=1))
        msk_sb = const.tile([P, P], MMDT, tag="msk")
        cos_sb = const.tile([P, (BT // P) * RD], F32, tag="cos")
        sin_sb = const.tile([P, (BT // P) * RD], F32, tag="sin")
        ident = const.tile([P, P], MMDT, tag="ident")
        wp_sb = const.tile([P, HPC * C], MMDT, tag="wp")
        ones_col = const.tile([P, 1], MMDT, tag="onec")
        ones_row = const.tile([1, P], MMDT, tag="oner")

        make_identity(nc, ident[:])
        nc.vector.memset(ones_col[:], 1.0)
        nc.vector.memset(ones_row[:], 1.0)

        qkt_pool = ep(tc.tile_pool(name="qkt", bufs=2))
        v_pool = ep(tc.tile_pool(name="v", bufs=2))
        yt_pool = ep(tc.tile_pool(name="yt", bufs=1))
        yt_all = yt_pool.tile([P, B * HPC * T], MMDT, tag="yt")
        xcol = ep(tc.tile_pool(name="xcol", bufs=4))
        rotp = ep(tc.tile_pool(name="rot", bufs=3))
        tmpp = ep(tc.tile_pool(name="tmp", bufs=2))
        attnp = ep(tc.tile_pool(name="attn", bufs=10))
        saccp = ep(tc.tile_pool(name="sacc", bufs=4))
        rowp = ep(tc.tile_pool(name="rows", bufs=4))
        rrepp = ep(tc.tile_pool(name="rrep", bufs=2))
        outrow = ep(tc.tile_pool(name="orow", bufs=3))

        # PSUM: exactly 8 banks
        ps5 = ep(tc.tile_pool(name="ps5", bufs=2, space=PSUM))   # qk gemm
        ps2 = ep(tc.tile_pool(name="ps2", bufs=1, space=PSUM))   # v gemm
        pst = ep(tc.tile_pool(name="pst", bufs=1, space=PSUM))   # transposes
        pss = ep(tc.tile_pool(name="pss", bufs=2, space=PSUM))   # scores
        psy = ep(tc.tile_pool(name="psy", bufs=2, space=PSUM))   # attn out

        # x chunk prefetching, one [P, KTC, P] chunk per A window
        prefetched = {}

        def fetch_x(b, m, split=1):
            key = (b, m)
            if key in prefetched:
                return prefetched.pop(key)
            xo = (b * MT + m) * KTC * P
            xc = xcol.tile([P, KTC, P], MMDT, tag="xc")
            # issue on the Act queue: the gpsimd queue carries the long
            # partition_all_reduce calls and must not delay x loads
            kc = KTC // split
            for s in range(split):
                nc.scalar.dma_start(
                    out=xc[:, s * kc:(s + 1) * kc, :],
                    in_=xt_d[:, xo + s * kc * P:xo + (s + 1) * kc * P]
                    .rearrange("p (k t) -> p k t", k=kc))
            return xc

        def prefetch_x(b, m, split=1):
            prefetched[(b, m)] = fetch_x(b, m, split=split)

        # first window split into 4 sub-DMAs so the first matmul only
        # waits for a quarter of the window (plus the first w chunk)
        prefetch_x(0, 0, split=4)

        wstack = ExitStack()
        wpool = wstack.enter_context(tc.tile_pool(name="wqkv", bufs=1))
        w_sbs = [wpool.tile([P, WQ], MMDT, tag=f"w{q}", name=f"w{q}")
                 for q in range(4)]
        # per-ktile weight chunks: each sweep matmul depends on a ~200KB
        # transfer, not a whole 0.8MB quarter — the DMA-bound first
        # windows stream instead of stalling at quarter boundaries
        for q in range(4):
            for c4 in range(4):
                nc.sync.dma_start(
                    out=w_sbs[q][:, c4 * FPC:(c4 + 1) * FPC],
                    in_=w_d[:, q * WQ + c4 * FPC:q * WQ + (c4 + 1) * FPC])
        # consts after the weights on the SP queue; cos/sin are consumed
        # one [P,RD] slice per window, so stream them in quarters to keep
        # them out of the startup HBM burst
        CS4 = (BT // P) * RD // 4
        for c4 in range(4):
            nc.sync.dma_start(out=cos_sb[:, c4 * CS4:(c4 + 1) * CS4],
                              in_=cos_d[:, c4 * CS4:(c4 + 1) * CS4])
            nc.sync.dma_start(out=sin_sb[:, c4 * CS4:(c4 + 1) * CS4],
                              in_=sin_d[:, c4 * CS4:(c4 + 1) * CS4])
            if c4 == 0:
                nc.sync.dma_start(out=msk_sb[:], in_=msk_d)
        nc.sync.dma_start(out=wp_sb[:], in_=wp_d)

        def wslice(kt, lo, hi):
            return w_sbs[kt // 4][:, (kt % 4) * FPC + lo:(kt % 4) * FPC + hi]

        qkts = {}
        v_sbs = {}
        # attention work broken into per-tile quanta, pumped one per
        # matmul slot inside the A windows / C tiles so the Scalar-bound
        # exp stream overlaps the PE-bound GEMMs instead of serializing
        quanta = deque()

        def pump(n=1):
            for _ in range(n):
                if not quanta:
                    return
                quanta.popleft()()

        # deferred per-window epilogues:
        # - V eviction of window g lands in window g+1 (depends only on
        #   the p2 psum, never on the Vector queue)
        # - the rot transposes of window g land in window g+2, so a full
        #   window of Vector-queue lag on RoPE never stalls the PE
        pending_v = [None]
        pending_t = deque()

        def flush_v():
            if pending_v[0] is None:
                return
            b, m, p2 = pending_v[0]
            pending_v[0] = None
            nc.scalar.copy(v_sbs[b][:, m * HPC * D:(m + 1) * HPC * D], p2[:])

        def flush_t(force=False):
            while pending_t and (force or len(pending_t) >= 2):
                b, m, rot = pending_t.popleft()
                tp4 = pst.tile([P, 4, P], MMDT, tag="tp", name=f"tp_{b}_{m}")
                for hb in range(4):
                    nc.tensor.transpose(tp4[:, hb, :],
                                        rot[:, hb * P:(hb + 1) * P], ident[:])
                qv = qkts[b][:].rearrange("p (hb t) -> p hb t", hb=4)
                nc.scalar.copy(qv[:, :, m * P:(m + 1) * P], tp4[:])

        def emit_a(b, m):
            if m == 0:
                qkts[b] = qkt_pool.tile([P, 4 * T], MMDT, tag="qkT",
                                        name=f"qkT_{b}")
                v_sbs[b] = v_pool.tile([P, MT * HPC * D], MMDT, tag="v",
                                       name=f"v_{b}")
            gi = b * MT + m
            # keep two windows of x in flight: the Act queue can lag
            # behind a pair's exp backlog
            for ahead in (1, 2):
                ni = gi + ahead
                if ni < B * MT and (ni // MT, ni % MT) not in prefetched \
                        and ni != gi:
                    prefetch_x(ni // MT, ni % MT)
            xc = fetch_x(b, m)
            p5 = ps5.tile([P, 512], F32, tag="p5", name=f"p5_{b}_{m}")
            p2 = ps2.tile([P, 256], F32, tag="p2", name=f"p2_{b}_{m}")
            # no pumps in this sweep: pumped score matmuls depend on the
            # qkT copies emitted by flush_t below, and must stay behind
            # them in the PE queue order
            for kt in range(KTC):
                nc.tensor.matmul(p5[:], xc[:, kt, :], wslice(kt, 0, 512),
                                 start=(kt == 0), stop=(kt == KTC - 1))
            flush_v()
            flush_t()
            # RoPE on the q|k half, writes rot
            gm = b * MT + m
            rot = rotp.tile([P, 512], MMDT, tag="rot", name=f"rot_{b}_{m}")
            p3 = p5[:].rearrange("p (blk two d) -> p blk two d", two=2, d=RD)
            re_, im_ = p3[:, :, 0, :], p3[:, :, 1, :]
            r3 = rot[:].rearrange("p (blk two d) -> p blk two d", two=2, d=RD)
            cosb = (cos_sb[:, gm * RD:(gm + 1) * RD]
                    .unsqueeze(1).broadcast_to([P, 4, RD]))
            sinb = (sin_sb[:, gm * RD:(gm + 1) * RD]
                    .unsqueeze(1).broadcast_to([P, 4, RD]))
            t1 = tmpp.tile([P, 256], F32, tag="t1")
            t2 = tmpp.tile([P, 256], F32, tag="t2")
            t1v = t1[:].rearrange("p (blk d) -> p blk d", d=RD)
            t2v = t2[:].rearrange("p (blk d) -> p blk d", d=RD)
            # the products must read the p5 PSUM, so they stay on Vector;
            # the combining SUB/ADD are SBUF->SBUF (and were the slow
            # strided-fp16-write ops), so they run on the otherwise-idle
            # gpsimd engine — this takes the Vector queue off the
            # window-seam critical path.
            nc.vector.tensor_tensor(t1v, re_, cosb, MUL)
            nc.vector.tensor_tensor(t2v, im_, sinb, MUL)
            nc.gpsimd.tensor_tensor(r3[:, :, 0, :], t1v, t2v, SUB)
            t3 = tmpp.tile([P, 256], F32, tag="t3")
            t4 = tmpp.tile([P, 256], F32, tag="t4")
            t3v = t3[:].rearrange("p (blk d) -> p blk d", d=RD)
            t4v = t4[:].rearrange("p (blk d) -> p blk d", d=RD)
            nc.vector.tensor_tensor(t3v, re_, sinb, MUL)
            nc.vector.tensor_tensor(t4v, im_, cosb, MUL)
            nc.gpsimd.tensor_tensor(r3[:, :, 1, :], t3v, t4v, ADD)
            # V projection sweep after the eviction so ps2 (bufs=1) is free
            for kt in range(KTC):
                nc.tensor.matmul(p2[:], xc[:, kt, :], wslice(kt, 512, FPC),
                                 start=(kt == 0), stop=(kt == KTC - 1))
                pump()
            pending_v[0] = (b, m, p2)
            pending_t.append((b, m, rot))

        # ---- attention chunk pairs (both heads), as pumpable quanta ----
        chunk_st = {}

        def enqueue_p(b, j):
            qkT = qkts[b]
            v_sb = v_sbs[b]
            nkt = 4 * (j + 1)
            st = {"y": {}, "sacc": {}, "at": {}}
            chunk_st[(b, j)] = st

            def q_score(kt, h, ktl, qlo):
                def f():
                    if kt == 0:
                        st["y"][h] = psy.tile([P, NQ], F32, tag="y",
                                              name=f"y_{b}_{j}_{h}")
                        st["sacc"][h] = saccp.tile([P, NQ], MMDT, tag="sa",
                                                   name=f"sa_{b}_{j}_{h}")
                    sacc = st["sacc"][h]
                    sc = pss.tile([P, NQ], F32, tag="sc",
                                  name=f"sc_{b}_{j}_{kt}_{h}")
                    nc.tensor.matmul(
                        sc[:, qlo:],
                        qkT[:, (2 + h) * T + kt * P:
                            (2 + h) * T + (kt + 1) * P],
                        qkT[:, h * T + j * NQ + qlo: h * T + (j + 1) * NQ],
                        start=True, stop=True)
                    at = attnp.tile([P, NQ], MMDT, tag="at",
                                    name=f"at_{b}_{j}_{kt}_{h}")
                    nc.scalar.activation(at[:, qlo:], sc[:, qlo:], EXP,
                                         scale=SCALE)
                    if ktl >= 0:
                        nc.vector.tensor_tensor(
                            at[:, qlo:qlo + P], at[:, qlo:qlo + P],
                            msk_sb[:], MUL)
                    if kt == 0:
                        nc.vector.tensor_copy(sacc[:], at[:])
                    else:
                        nc.vector.tensor_tensor(sacc[:, qlo:], sacc[:, qlo:],
                                                at[:, qlo:], ADD)
                    st["at"][(kt, h)] = at
                return f

            def q_v(kt, h, qlo):
                def f():
                    at = st["at"].pop((kt, h))
                    nc.tensor.matmul(
                        st["y"][h][:, qlo:],
                        v_sb[:, kt * HPC * D + h * D:
                             kt * HPC * D + (h + 1) * D],
                        at[:, qlo:], start=(kt == 0), stop=(kt == nkt - 1),
                        skip_group_check=True)
                return f

            def q_f1(h):
                # softmax denominator via the PE: a ones-column matmul
                # sums sacc over its key partitions into a [1,NQ] psum
                # row (213ns) — unlike the 3.5us gpsimd
                # partition_all_reduce, this never turns into a long
                # cross-engine stall when the list scheduler hoists the
                # downstream reciprocal
                def f():
                    st.setdefault("rrep", {})
                    prow = pss.tile([P, NQ], F32, tag="sc",
                                    name=f"dsum_{b}_{j}_{h}")
                    nc.tensor.matmul(prow[0:1, :], ones_col[:],
                                     st["sacc"][h][:], start=True, stop=True)
                    srow = rowp.tile([1, NQ], F32, tag="dr",
                                     name=f"drow_{b}_{j}_{h}")
                    nc.scalar.copy(srow[:], prow[0:1, :])
                    rrow32 = rowp.tile([1, NQ], F32, tag="rrow32",
                                       name=f"rrow32_{b}_{j}_{h}")
                    with nc.allow_low_precision(reason="softmax recip"):
                        nc.vector.reciprocal_approx_fast(out=rrow32[:],
                                                         in_=srow[:])
                    rrow = rowp.tile([1, NQ], MMDT, tag="rrow",
                                     name=f"rrow_{b}_{j}_{h}")
                    nc.scalar.copy(rrow[:], rrow32[:])
                    # DRAM bounce: broadcast the [1,NQ] reciprocal row
                    # across partitions (SBUF APs cannot have zero
                    # partition step, DRAM APs can). Done here, windows
                    # before F2 consumes it, so the DMA latency never
                    # touches the psy bank handoff.
                    ri = (b * NJ + j) * HPC + h
                    nc.scalar.dma_start(out=rbc_d[ri:ri + 1, :], in_=rrow[:])
                    rrep = rrepp.tile([P, NQ], MMDT, tag="rr",
                                      name=f"rr_{b}_{j}_{h}")
                    nc.scalar.dma_start(
                        out=rrep[:],
                        in_=rbc_d[ri:ri + 1, :].broadcast_to([P, NQ]))
                    st["rrep"][h] = rrep
                return f

            # the v matmuls trail their score quanta by VLAG k-tiles: the
            # exp -> mask -> sacc chain for a tile then has ~2.5us of
            # slack before the PE's y matmul needs the at tile, so a
            # backlogged Scalar queue no longer stalls the PE
            VLAG = 3

            def qlo_of(kt):
                return max(kt - (nkt - 4), 0) * P

            for kt in range(nkt):
                ktl = kt - (nkt - 4)
                qlo = qlo_of(kt)
                for h in range(HPC):
                    quanta.append(q_score(kt, h, ktl, qlo))
                if kt >= VLAG:
                    for h in range(HPC):
                        quanta.append(q_v(kt - VLAG, h, qlo_of(kt - VLAG)))
            for kt in range(max(nkt - VLAG, 0), nkt):
                for h in range(HPC):
                    quanta.append(q_v(kt, h, qlo_of(kt)))
            for h in range(HPC):
                quanta.append(q_f1(h))

        def emit_f2(b, j):
            # normalization: reciprocal on the [1,NQ] denominator row,
            # broadcast across partitions with a rank-1 ones outer
            # product on the PE, then scale y. Every link is a short op
            # with multi-window slack, so nothing here can stall a queue.
            st = chunk_st.pop((b, j))
            inst0 = b * HPC
            for h in range(HPC):
                rrep = st["rrep"][h]
                nc.vector.tensor_tensor(
                    yt_all[:, (inst0 + h) * T + j * NQ:
                           (inst0 + h) * T + (j + 1) * NQ],
                    st["y"][h][:], rrep[:], MUL)

        # ---- output projection m-tile (reuses the A/B psum banks) ----
        c_cnt = [0]

        def emit_c(b, m):
            orow = outrow.tile([P, C], MMDT, tag="orow")
            for oc in range(4):
                pool, tg = (ps5, "p5") if c_cnt[0] % 2 == 0 else (pss, "sc")
                c_cnt[0] += 1
                op = pool.tile([P, 512], F32, tag=tg, name=f"op_{b}_{m}_{oc}")
                for h in range(HPC):
                    nc.tensor.matmul(
                        op[:],
                        yt_all[:, (b * HPC + h) * T + m * P:
                               (b * HPC + h) * T + (m + 1) * P],
                        wp_sb[:, h * C + oc * 512: h * C + (oc + 1) * 512],
                        start=(h == 0), stop=(h == HPC - 1))
                if oc < 3:
                    pump()
                # single-shot [P,512] evictions, balanced so Scalar keeps
                # headroom for the last pair's exp stream: Vector takes
                # oc0, oc1 and the tail half of oc2; Scalar takes oc3 and
                # the head half of oc2
                if oc in (0, 1):
                    nc.vector.tensor_copy(
                        orow[:, oc * 512:(oc + 1) * 512], op[:])
                elif oc == 2:
                    nc.scalar.copy(orow[:, oc * 512:oc * 512 + 256],
                                   op[:, 0:256])
                    nc.vector.tensor_copy(
                        orow[:, oc * 512 + 256:(oc + 1) * 512],
                        op[:, 256:512])
                else:
                    nc.scalar.copy(orow[:, oc * 512:(oc + 1) * 512], op[:])
                if b == 1 and m >= 14:
                    # last tiles: per-oc DMA chunks so the final transfer
                    # overlaps the remaining evictions instead of
                    # lengthening the tail
                    nc.sync.dma_start(
                        out=out_d[(b * MT + m) * P:(b * MT + m + 1) * P,
                                  oc * 512:(oc + 1) * 512],
                        in_=orow[:, oc * 512:(oc + 1) * 512])
            if not (b == 1 and m >= 14):
                nc.sync.dma_start(
                    out=out_d[(b * MT + m) * P:(b * MT + m + 1) * P, :],
                    in_=orow[:])

        # ---- the fused schedule ----
        # pair j's quanta (including its two trailing F1 reduces) enqueue
        # after window g_P = 16b+4(j+1) and drain one per p2-loop matmul
        # slot in windows g_P+1..g_P+4 (16 slots each, 16j+18 quanta). F2
        # lands at window g_P+8: its Vector-queue reciprocal must only run
        # after the 3.5us gpsimd reduces, or it stalls the RoPE ops queued
        # behind it (which the flush_t transposes wait on).
        f2_at = {}
        for b in range(B):
            for j in range(4):
                f2_at.setdefault(b * MT + 4 * (j + 1) + 8, []).append((b, j))
        sched = []
        for b in range(B):
            for m in range(MT):
                g = b * MT + m
                for (fb, fj) in f2_at.get(g, []):
                    sched.append(("F2", fb, fj))
                sched.append(("A", b, m))
                if m in (4, 8, 12):
                    sched.append(("P", b, m // 4 - 1))
                if b > 0 and m == 0:
                    sched.append(("P", b - 1, 3))
        bl = B - 1

        emitters = {"A": emit_a, "P": enqueue_p, "F2": emit_f2, "C": emit_c}
        for kind, b, i in sched:
            emitters[kind](b, i)

        # ---- tail: flush the last epilogues, drain pair (bl,3) through
        # the C tiles' pump slots ----
        flush_v()
        flush_t(force=True)
        emit_f2(bl, 1)          # g_P+8 = 32 lands here
        enqueue_p(bl, 3)
        emit_c(0, 0)
        emit_f2(bl, 2)          # its F1 reduces drained in C(0,0)'s pumps
        for m in range(1, MT):
            emit_c(0, m)
        for m in range(0, 8):
            emit_c(1, m)
        pump(len(quanta))       # leftovers of (bl,3), incl. its F1s
        emit_f2(bl, 3)
        for m in range(8, MT):
            emit_c(1, m)

        wstack.close()

    nc.compile()
    return nc


def _perm(rows):
    return np.concatenate([rows[0::2], rows[1::2]], axis=0)


def _host_inputs(x, mask, freqs_cos, freqs_sin, w_attn, w_proj):
    f32 = np.float32
    f16 = np.float16
    x = np.asarray(x, f32)
    fc = np.asarray(freqs_cos, f32)
    fs = np.asarray(freqs_sin, f32)
    w_attn = np.asarray(w_attn, f32)
    w_proj = np.asarray(w_proj, f32)

    # x in DMA-issue order: per partition, contiguous [b][m][kt][tok]
    Xv = x.reshape(B, MT, P, KTC, P).transpose(4, 0, 1, 3, 2)
    # [p, b, m, kt, tok]
    xt_host = np.ascontiguousarray(Xv.reshape(P, -1)).astype(f16)

    def rows_arrange(a):  # [BT, RD] -> [P, (BT//P)*RD]
        return np.ascontiguousarray(
            a.reshape(BT // P, P, RD).transpose(1, 0, 2).reshape(P, -1))

    cosw = rows_arrange(np.concatenate([fc] * B, axis=0))
    sinw = rows_arrange(np.concatenate([fs] * B, axis=0))

    # one [k, q] triangle (attend iff k <= q) covers every diagonal subtile
    maskd = np.ascontiguousarray(np.triu(np.ones((P, P), dtype=f16)))

    wq, wk, wv = w_attn[0:C], w_attn[C:2 * C], w_attn[2 * C:3 * C]
    in_maps = []
    for c in range(NCORES):
        h0, h1 = HPC * c, HPC * c + 1
        Wc = np.concatenate([
            _perm(wq[h0 * D:(h0 + 1) * D]), _perm(wq[h1 * D:(h1 + 1) * D]),
            _perm(wk[h0 * D:(h0 + 1) * D]), _perm(wk[h1 * D:(h1 + 1) * D]),
            wv[h0 * D:(h0 + 1) * D], wv[h1 * D:(h1 + 1) * D]], axis=0)
        wqkv_c = np.ascontiguousarray(
            Wc.T.reshape(KTC, P, FPC).transpose(1, 0, 2).reshape(P, KTC * FPC)
        ).astype(f16)
        wp_c = w_proj[:, c * HPC * D:(c + 1) * HPC * D].T  # [256, C]
        wp_c = np.ascontiguousarray(
            wp_c.reshape(HPC, P, C).transpose(1, 0, 2).reshape(P, HPC * C)
        ).astype(f16)
        in_maps.append({
            "xt": xt_host, "wqkv": wqkv_c, "cosw": cosw, "sinw": sinw,
            "maskd": maskd, "wproj": wp_c,
        })
    return in_maps


def kernel(x, mask, freqs_cos, freqs_sin, w_attn, w_proj):
    global _PROGRAM
    _ensure_concourse()
    from concourse.bass_utils import run_bass_kernel_spmd

    if _PROGRAM is None:
        _PROGRAM = _build_program()
    nc = _PROGRAM

    in_maps = _host_inputs(x, mask, freqs_cos, freqs_sin, w_attn, w_proj)
    res = run_bass_kernel_spmd(nc, in_maps, list(range(NCORES)))
    out = res.results[0]["outp"].astype(np.float32)
    for i in range(1, NCORES):
        out = out + res.results[i]["outp"].astype(np.float32)
    return np.ascontiguousarray(out.reshape(B, T, C))

